# revision 17
# baseline (speedup 1.0000x reference)
"""Bass/Trainium2 kernel for nn_HNO_37065567764989 (self-contained).

Strategy (8 NeuronCores, SPMD):
- Branch matvec b = Wb@a column-sharded 8 ways. Each core streams its 16MB
  shard as fp16 (W scaled by 2^10; a as an fp16 hi/lo stationary pair), two
  512KB DMAs per 1MB chunk across queues. 512B AllReduce combines partials.
- Nx=32768 points sharded 8 ways (4096/core). Trunk runs as 4 wide pairs
  (tiles f and f+4 share [128,1024] elementwise ops that write the energy
  movings directly). GpSimd carries only early-pair products plus the
  collective, so the mesh wait never blocks the trunk tail.
- EnergyNet first layer uses runtime outer-product stationaries S=c(x)p,
  c(x)q built on-device after the AllReduce -- no per-row extraction.
- Precision: t2/P1 flow as fp16 hi/lo pairs; t1/tp1, B/C stationaries and
  all product chains are single fp16 (mirror-validated 1.24e-2).
"""
import sys

for _p in ("/opt/trn_rl_repo",):
    if _p not in sys.path:
        sys.path.insert(0, _p)

import numpy as np

MP1, NX, P, HT, HE = 524288, 32768, 128, 128, 64
NCORES = 8
KSH = MP1 // NCORES        # 65536 contraction elems per core
NKT = KSH // 128           # 512 k-tiles
NCHUNK = 16
KTC = NKT // NCHUNK        # 32 k-tiles per chunk
NPTS = NX // NCORES        # 4096 points per core
FD = 512
WFD = 2 * FD               # wide pair width
NTRUNK = NPTS // FD        # 8 trunk tiles
NEN = NTRUNK // 2          # 4 energy tiles / trunk pairs

_PKB = {"wt2h": 0, "wt2l": 128, "w2ah": 256, "w2al": 384, "w2b": 512,
        "w2c": 640}
PKB_COLS = 768
_PKC = {"wt3h": 0, "wt3l": 128, "e0": 256, "eq": 384, "ep": 512,
        "pq2": 640, "v6": 768}
PKC_COLS = 774
_PK32 = {"c1b": 0, "bt2b": 1, "be1b2": 2, "be2b2": 3, "sel4m": 4}
PK32_COLS = 8

_CACHE = {}


def _build():
    import concourse.bacc as bacc
    import concourse.mybir as mybir
    from concourse import tile

    f32 = mybir.dt.float32
    f16 = mybir.dt.float16
    AF = mybir.ActivationFunctionType
    ALU = mybir.AluOpType

    nc = bacc.Bacc("TRN2", target_bir_lowering=False, debug=False,
                   num_devices=NCORES)

    w_d = nc.dram_tensor("w", [NCHUNK, 128, KTC * 128], f16, kind="ExternalInput")
    a_d = nc.dram_tensor("a2", [128, NKT, 2], f16, kind="ExternalInput")
    x_d = nc.dram_tensor("x4", [4, NPTS], f16, kind="ExternalInput")
    w11_d = nc.dram_tensor("w11", [4, 128], f16, kind="ExternalInput")
    pkb_d = nc.dram_tensor("pkb", [128, PKB_COLS], f16, kind="ExternalInput")
    pkc_d = nc.dram_tensor("pkc", [128, PKC_COLS], f16, kind="ExternalInput")
    pk32_d = nc.dram_tensor("pk32", [128, PK32_COLS], f32, kind="ExternalInput")
    out_d = nc.dram_tensor("out", [2, NPTS // 2], f32, kind="ExternalOutput")
    cc_in = nc.dram_tensor("cc_in", [128, 1], f32)
    cc_out = nc.dram_tensor("cc_out", [128, 1], f32, addr_space="Shared")

    def TT(eng, out, i0, i1, op=ALU.mult):
        eng.tensor_tensor(out, i0, i1, op)

    with tile.TileContext(nc) as tc:
        with (
            tc.tile_pool(name="smp", bufs=1) as smp,
            tc.tile_pool(name="persist", bufs=1) as persist,
            tc.tile_pool(name="wpool", bufs=4) as wpool,
            tc.tile_pool(name="scr", bufs=1) as scr,
            tc.tile_pool(name="ps8", bufs=1, space="PSUM") as ps,
        ):
            # ---- packed constant loads (6 DMA issues) ----
            x4 = smp.tile([4, NPTS], f16, name="x4t")
            nc.sync.dma_start(x4[:], x_d.ap())
            w11 = smp.tile([4, 128], f16, name="w11t")
            nc.sync.dma_start(w11[:], w11_d.ap())
            pk32 = smp.tile([128, PK32_COLS], f32, name="pk32t")
            nc.sync.dma_start(pk32[:], pk32_d.ap())
            a2 = smp.tile([128, NKT, 2], f16, name="a2t")
            nc.sync.dma_start(a2[:], a_d.ap())
            pkb = smp.tile([128, PKB_COLS], f16, name="pkbt")
            nc.sync.dma_start(pkb[:], pkb_d.ap())
            pkc = smp.tile([128, PKC_COLS], f16, name="pkct")
            nc.sync.dma_start(pkc[:], pkc_d.ap())

            smt = {"w11": w11[:]}
            for n_, c0 in _PKB.items():
                smt[n_] = pkb[:, c0:c0 + 128]
            for n_, c0 in _PKC.items():
                if n_ == "v6":
                    smt[n_] = pkc[:, c0:c0 + 6]
                elif n_ == "pq2":
                    smt[n_] = pkc[0:1, c0:c0 + 128]
                else:
                    smt[n_] = pkc[:, c0:c0 + 128]
            for n_, c0 in _PK32.items():
                if n_ == "sel4m":
                    smt[n_] = pk32[0:8, c0:c0 + 4]
                else:
                    smt[n_] = pk32[:, c0:c0 + 1]

            # ---- trunk layer-1 z1 matmuls (pairs f, f+4) ----
            # z1 borrows pB/pC/pBC banks so zA/zB/aA/aB stay free for the
            # layer-2 wave to start as soon as l1 of pair 0 is done.
            zpair = [("zA", "zB"), ("aA", "aB")]
            z1tags = ["pB", "pC", "pBC"]
            z1ps = {}
            zi = 0
            for j in range(NEN):
                for hx, f in enumerate((j, j + 4)):
                    cs = slice(f * FD, (f + 1) * FD)
                    z1 = ps.tile([128, FD], f32, tag=z1tags[zi % 3], name=f"z1_{f}")
                    zi += 1
                    nc.tensor.matmul(z1[:], smt["w11"], x4[:, cs], start=True,
                                     stop=True)
                    z1ps[f] = z1

            # ---- trunk layer-1 elementwise (wide pairs, single-fp16 t1/tp1) --
            l1 = {}
            for j in range(NEN):
                t1f = scr.tile([128, WFD], f32, tag="t1f", name=f"t1f_{j}")
                for hx, f in enumerate((j, j + 4)):
                    hs = slice(hx * FD, (hx + 1) * FD)
                    nc.scalar.activation(t1f[:, hs], z1ps[f][:], AF.Tanh,
                                         bias=smt["c1b"])
                t1h = persist.tile([128, WFD], f16, tag=f"t1h_{j % 2}", name=f"t1h_{j}")
                nc.scalar.copy(t1h[:], t1f[:])
                s1 = scr.tile([128, WFD], f32, tag="s1", name=f"s1_{j}")
                nc.scalar.square(s1[:], t1f[:])
                tp1f = scr.tile([128, WFD], f32, tag="tp1f", name=f"tp1f_{j}")
                nc.scalar.activation(tp1f[:], s1[:], AF.Copy, bias=1.0, scale=-1.0)
                tp1h = persist.tile([128, WFD], f16, tag=f"tp1h_{j % 2}", name=f"tp1h_{j}")
                nc.scalar.copy(tp1h[:], tp1f[:])
                g2m = persist.tile([128, WFD], f16, tag=f"g2m_{j % 2}", name=f"g2m_{j}")
                TT(nc.vector, g2m[:], t1h[:], tp1h[:])
                # g3m = (tp1-2/3)*tp1 = -(s1-1/3)*tp1; sign folded into H below
                g3m = persist.tile([128, WFD], f16, tag=f"g3m_{j % 2}", name=f"g3m_{j}")
                nc.vector.scalar_tensor_tensor(
                    g3m[:], tp1h[:], 2.0 / 3.0, tp1h[:], ALU.subtract, ALU.mult)
                l1[j] = (t1h, tp1h, g2m, g3m)

            # ---- trunk layer-2 wave (wide pairs) ----
            sh = {}
            for j in range(NEN):
                t1h, tp1h, g2m, g3m = l1[j]
                shj = tuple(
                    persist.tile([128, WFD], f16, tag=f"sh{nm}_{j}",
                                 name=f"sh_{nm}_{j}")
                    for nm in ("t2h", "t2l", "P1h", "P1l", "ux2", "ux3"))
                t2h_s, t2l_s, P1h_s, P1l_s, ux2_s, ux3_s = shj
                sh[j] = shj
                ge = nc.gpsimd if j < 2 else nc.vector

                zw, aw = [], []
                for hx in range(2):
                    hs = slice(hx * FD, (hx + 1) * FD)
                    z2 = ps.tile([128, FD], f32, tag=zpair[0][hx], name=f"z2_{j}{hx}")
                    nc.tensor.matmul(z2[:], smt["wt2h"], t1h[:, hs], start=True,
                                     stop=False)
                    nc.tensor.matmul(z2[:], smt["wt2l"], t1h[:, hs], start=False,
                                     stop=True)
                    zw.append(z2)
                    A = ps.tile([128, FD], f32, tag=zpair[1][hx], name=f"A_{j}{hx}")
                    nc.tensor.matmul(A[:], smt["w2ah"], tp1h[:, hs], start=True,
                                     stop=False)
                    nc.tensor.matmul(A[:], smt["w2al"], tp1h[:, hs], start=False,
                                     stop=True)
                    aw.append(A)

                Bc = scr.tile([128, WFD], f16, tag="Bc", name=f"Bc_{j}")
                Cc = scr.tile([128, WFD], f16, tag="Cc", name=f"Cc_{j}")
                A2c = scr.tile([128, WFD], f16, tag="A2c", name=f"A2c_{j}")
                Acp = scr.tile([128, WFD], f16, tag="Acp", name=f"Acp_{j}")
                t2f = scr.tile([128, WFD], f32, tag="t2f", name=f"t2f_{j}")
                for hx in range(2):
                    hs = slice(hx * FD, (hx + 1) * FD)
                    nc.scalar.activation(t2f[:, hs], zw[hx][:], AF.Tanh,
                                         bias=smt["bt2b"])
                    nc.scalar.square(A2c[:, hs], aw[hx][:])
                    nc.scalar.copy(Acp[:, hs], aw[hx][:])
                    B = ps.tile([128, FD], f32, tag="pB", name=f"B_{j}{hx}")
                    nc.tensor.matmul(B[:], smt["w2b"], g2m[:, hs], start=True,
                                     stop=True)
                    nc.scalar.copy(Bc[:, hs], B[:])
                    C = ps.tile([128, FD], f32, tag="pC", name=f"C_{j}{hx}")
                    nc.tensor.matmul(C[:], smt["w2c"], g3m[:, hs], start=True,
                                     stop=True)
                    nc.scalar.copy(Cc[:, hs], C[:])

                nc.scalar.copy(t2h_s[:], t2f[:])
                TT(nc.gpsimd, t2l_s[:], t2f[:], t2h_s[:], ALU.subtract)
                s2 = scr.tile([128, WFD], f32, tag="s2", name=f"s2_{j}")
                nc.scalar.square(s2[:], t2f[:])
                tp2 = scr.tile([128, WFD], f32, tag="tp2", name=f"tp2_{j}")
                nc.vector.tensor_scalar(tp2[:], s2[:], -1.0, 1.0, ALU.mult, ALU.add)
                tp2c = scr.tile([128, WFD], f16, tag="tp2c", name=f"tp2c_{j}")
                nc.scalar.activation(tp2c[:], s2[:], AF.Copy, bias=1.0, scale=-1.0)
                P1f = scr.tile([128, WFD], f32, tag="P1f", name=f"P1f_{j}")
                for hx in range(2):
                    hs = slice(hx * FD, (hx + 1) * FD)
                    TT(nc.vector, P1f[:, hs], tp2[:, hs], aw[hx][:])
                nc.scalar.copy(P1h_s[:], P1f[:])
                TT(nc.gpsimd, P1l_s[:], P1f[:], P1h_s[:], ALU.subtract)

                T1 = scr.tile([128, WFD], f16, tag="T1", name=f"T1_{j}")
                TT(ge, T1[:], t2h_s[:], A2c[:])
                E = scr.tile([128, WFD], f16, tag="E", name=f"E_{j}")
                nc.vector.scalar_tensor_tensor(
                    E[:], T1[:], -2.0, Bc[:], ALU.mult, ALU.add)
                TT(nc.vector, ux2_s[:], tp2c[:], E[:])
                A3 = scr.tile([128, WFD], f16, tag="A3", name=f"A3_{j}")
                TT(ge, A3[:], A2c[:], Acp[:])
                G1 = scr.tile([128, WFD], f16, tag="G1", name=f"G1_{j}")
                nc.vector.scalar_tensor_tensor(
                    G1[:], tp2c[:], 2.0 / 3.0, A3[:], ALU.subtract, ALU.mult)
                G2 = scr.tile([128, WFD], f16, tag="G2", name=f"G2_{j}")
                TT(ge, G2[:], t2h_s[:], Acp[:])
                G3 = scr.tile([128, WFD], f16, tag="G3", name=f"G3_{j}")
                TT(ge, G3[:], G2[:], Bc[:])
                D = scr.tile([128, WFD], f16, tag="Dd", name=f"D_{j}")
                TT(nc.vector, D[:], G1[:], G3[:], ALU.add)
                # Cc holds -C_true (g3m sign-flip): H = -6*D - Cc = -6*D + C
                H = scr.tile([128, WFD], f16, tag="Hh", name=f"H_{j}")
                nc.vector.scalar_tensor_tensor(
                    H[:], D[:], -6.0, Cc[:], ALU.mult, ALU.subtract)
                TT(nc.vector, ux3_s[:], tp2c[:], H[:])

            # ---- matvec: stream W shard (2 DMA splits per 1MB chunk) ----
            b8 = ps.tile([8, FD], f32, tag="pMV", name="b8")
            half = KTC * 64
            for i in range(NCHUNK):
                wch = wpool.tile([128, KTC * 128], f16, tag="wch", name="wch")
                nc.sync.dma_start(wch[:, 0:half], w_d.ap()[i][:, 0:half])
                nc.sync.dma_start(wch[:, half:], w_d.ap()[i][:, half:])
                for g in range(KTC // 4):
                    nc.tensor.matmul(
                        b8[:], a2[:, i * KTC + 4 * g:i * KTC + 4 * (g + 1), :],
                        wch[:, g * 512:(g + 1) * 512],
                        start=(i == 0 and g == 0),
                        stop=(i == NCHUNK - 1 and g == KTC // 4 - 1),
                    )

            # ---- local reduce + AllReduce (high priority: the mesh gates
            # the whole energy phase, so these must not sit behind the
            # trunk wave in the engine queues) ----
            with tc.high_priority():
                b8sb = smp.tile([8, FD], f32, name="b8sb")
                nc.vector.tensor_copy(b8sb[:], b8[:])
                bcol = ps.tile([128, 1], f32, tag="pBC", name="bcol")
                for j in range(4):
                    nc.tensor.matmul(bcol[:], b8sb[:, j * 128:(j + 1) * 128],
                                     smt["sel4m"][:, j:j + 1],
                                     start=(j == 0), stop=(j == 3))
                b_loc = smp.tile([128, 1], f32, name="bloc")
                nc.vector.tensor_copy(b_loc[:], bcol[:])
                nc.sync.dma_start(cc_in.ap(), b_loc[:])
                nc.gpsimd.collective_compute(
                    "AllReduce", ALU.add,
                    replica_groups=[list(range(NCORES))],
                    ins=[cc_in.ap()], outs=[cc_out.ap()],
                )
                b_ar = smp.tile([128, 1], f32, name="bar")
                nc.sync.dma_start(b_ar[:], cc_out.ap())

            # ---- b -> c -> outer-product stationaries S = c(x)p, c(x)q ----
            with tc.high_priority():
                b16 = smp.tile([128, 1], f16, name="b16")
                nc.vector.tensor_copy(b16[:], b_ar[:])
                c0p = ps.tile([1, 128], f32, tag="pBC", name="c0p")
                nc.tensor.matmul(c0p[:], b16[:], smt["wt3h"], start=True, stop=False)
                nc.tensor.matmul(c0p[:], b16[:], smt["wt3l"], start=False, stop=True)
                c0sb = smp.tile([1, 128], f16, name="c0sb")
                nc.vector.tensor_copy(c0sb[:], c0p[:])
                scpq_p = ps.tile([128, 128], f32, tag="pBC", name="scpq_p")
                nc.tensor.matmul(scpq_p[:], c0sb[:], smt["pq2"], start=True, stop=True)
                Scpq16 = smp.tile([128, 128], f16, name="Scpq16")
                nc.scalar.copy(Scpq16[:], scpq_p[:])
            Sp16, Sq16 = Scpq16[:, 0:64], Scpq16[:, 64:128]

            # ---- energy phase ----
            for e in range(NEN):
                t2h_s, t2l_s, P1h_s, P1l_s, ux2_s, ux3_s = sh[e]
                trio = [["zA", "zB", "aA"], ["aB", "pB", "pC"]][e % 2]
                dzt, dyt = ("pBC", "pMV") if e % 2 == 0 else ("pMV", "pBC")

                z1e = ps.tile([128, FD], f32, tag=trio[0], name=f"z1e_{e}")
                z1p = ps.tile([128, FD], f32, tag=trio[1], name=f"z1p_{e}")
                z1pp = ps.tile([128, FD], f32, tag=trio[2], name=f"z1pp_{e}")
                for hx in range(2):
                    hs = slice(hx * FD, (hx + 1) * FD)
                    rs = slice(hx * 64, (hx + 1) * 64)
                    nc.tensor.matmul(z1e[rs, :], Sp16, t2h_s[:, hs], start=True,
                                     stop=False)
                    nc.tensor.matmul(z1e[rs, :], Sp16, t2l_s[:, hs], start=False,
                                     stop=False)
                    nc.tensor.matmul(z1e[rs, :], Sq16, P1h_s[:, hs], start=False,
                                     stop=False)
                    nc.tensor.matmul(z1e[rs, :], Sq16, P1l_s[:, hs], start=False,
                                     stop=True)
                    nc.tensor.matmul(z1p[rs, :], Sp16, P1h_s[:, hs], start=True,
                                     stop=False)
                    nc.tensor.matmul(z1p[rs, :], Sp16, P1l_s[:, hs], start=False,
                                     stop=False)
                    nc.tensor.matmul(z1p[rs, :], Sq16, ux2_s[:, hs], start=False,
                                     stop=True)
                    nc.tensor.matmul(z1pp[rs, :], Sp16, ux2_s[:, hs], start=True,
                                     stop=False)
                    nc.tensor.matmul(z1pp[rs, :], Sq16, ux3_s[:, hs], start=False,
                                     stop=True)

                t1ef = scr.tile([128, FD], f32, tag="t1ef", name=f"t1ef_{e}")
                nc.scalar.activation(t1ef[:], z1e[:], AF.Tanh, bias=smt["be1b2"])
                t1eh = scr.tile([128, FD], f16, tag="t1eh", name=f"t1eh_{e}")
                nc.scalar.copy(t1eh[:], t1ef[:])
                z1psb = scr.tile([128, FD], f16, tag="z1psb", name=f"z1psb_{e}")
                nc.scalar.copy(z1psb[:], z1p[:])
                z1ppsb = scr.tile([128, FD], f16, tag="z1ppsb", name=f"z1ppsb_{e}")
                nc.scalar.copy(z1ppsb[:], z1pp[:])
                s1e = scr.tile([128, FD], f16, tag="s1e", name=f"s1e_{e}")
                nc.scalar.square(s1e[:], t1ef[:])
                m_ = scr.tile([128, FD], f16, tag="m_", name=f"m_{e}")
                nc.scalar.activation(m_[:], s1e[:], AF.Copy, bias=1.0, scale=-1.0)
                z1p2 = scr.tile([128, FD], f16, tag="z1p2", name=f"z1p2_{e}")
                TT(nc.gpsimd, z1p2[:], z1psb[:], z1psb[:])
                N1 = scr.tile([128, FD], f16, tag="N1", name=f"N1_{e}")
                TT(nc.vector, N1[:], t1eh[:], m_[:])
                a1p = scr.tile([128, FD], f16, tag="a1p", name=f"a1p_{e}")
                TT(nc.vector, a1p[:], m_[:], z1psb[:])
                N2 = scr.tile([128, FD], f16, tag="N2", name=f"N2_{e}")
                TT(nc.gpsimd, N2[:], N1[:], z1p2[:])
                N3 = scr.tile([128, FD], f16, tag="N3", name=f"N3_{e}")
                TT(nc.vector, N3[:], m_[:], z1ppsb[:])
                zin = scr.tile([128, FD], f16, tag="zin", name=f"zin_{e}")
                nc.vector.scalar_tensor_tensor(
                    zin[:], N2[:], -2.0, N3[:], ALU.mult, ALU.add)
                mpc = scr.tile([128, FD], f16, tag="mpc", name=f"mpc_{e}")
                TT(nc.vector, mpc[:], N1[:], z1psb[:])
                O1 = scr.tile([128, FD], f16, tag="O1", name=f"O1_{e}")
                nc.vector.scalar_tensor_tensor(
                    O1[:], s1e[:], 1.0 / 3.0, m_[:], ALU.subtract, ALU.mult)
                O2f = scr.tile([128, FD], f16, tag="O2f", name=f"O2f_{e}")
                TT(nc.gpsimd, O2f[:], O1[:], z1p2[:])
                O3f = scr.tile([128, FD], f16, tag="O3f", name=f"O3f_{e}")
                TT(nc.vector, O3f[:], N1[:], z1ppsb[:])
                O2m = scr.tile([128, FD], f16, tag="O2m", name=f"O2m_{e}")
                nc.vector.scalar_tensor_tensor(
                    O2m[:], O2f[:], 3.0, O3f[:], ALU.mult, ALU.subtract)

                z2e = ps.tile([128, FD], f32, tag=trio[0], name=f"z2e_{e}")
                nc.tensor.matmul(z2e[:], smt["e0"], t1eh[:], start=True, stop=True)
                z2ep = ps.tile([128, FD], f32, tag=trio[1], name=f"z2ep_{e}")
                nc.tensor.matmul(z2ep[:], smt["e0"], a1p[:], start=True, stop=True)
                z2epp = ps.tile([128, FD], f32, tag=trio[2], name=f"z2epp_{e}")
                nc.tensor.matmul(z2epp[:], smt["e0"], zin[:], start=True, stop=True)
                Dz = ps.tile([128, FD], f32, tag=dzt, name=f"Dz_{e}")
                nc.tensor.matmul(Dz[:], smt["eq"], m_[:], start=True, stop=True)
                DyN = ps.tile([128, FD], f32, tag=dyt, name=f"DyN_{e}")
                nc.tensor.matmul(DyN[:], smt["ep"], m_[:], start=True, stop=True)
                DzpN = ps.tile([128, FD], f32, tag=trio[0], name=f"DzpN_{e}")
                nc.tensor.matmul(DzpN[:], smt["eq"], mpc[:], start=True, stop=True)
                DypN = ps.tile([128, FD], f32, tag=trio[1], name=f"DypN_{e}")
                nc.tensor.matmul(DypN[:], smt["ep"], mpc[:], start=True, stop=True)
                Dzpp2 = ps.tile([128, FD], f32, tag=trio[2], name=f"Dzpp2_{e}")
                nc.tensor.matmul(Dzpp2[:], smt["eq"], O2m[:], start=True, stop=True)

                t2e = scr.tile([128, FD], f16, tag="t2e", name=f"t2e_{e}")
                nc.scalar.activation(t2e[:], z2e[:], AF.Tanh, bias=smt["be2b2"])
                s2e = scr.tile([128, FD], f16, tag="s2e", name=f"s2e_{e}")
                nc.scalar.square(s2e[:], t2e[:])
                w_ = scr.tile([128, FD], f16, tag="w_", name=f"w_{e}")
                nc.scalar.activation(w_[:], s2e[:], AF.Copy, bias=1.0, scale=-1.0)
                z2ep16 = scr.tile([128, FD], f16, tag="z2ep16", name=f"z2ep16_{e}")
                nc.scalar.copy(z2ep16[:], z2ep[:])
                z2ep2 = scr.tile([128, FD], f16, tag="z2ep2", name=f"z2ep2_{e}")
                TT(nc.gpsimd, z2ep2[:], z2ep16[:], z2ep16[:])
                Q1 = scr.tile([128, FD], f16, tag="Q1", name=f"Q1_{e}")
                TT(nc.vector, Q1[:], t2e[:], w_[:])
                wpc = scr.tile([128, FD], f16, tag="wpc", name=f"wpc_{e}")
                TT(nc.vector, wpc[:], Q1[:], z2ep16[:])
                R1 = scr.tile([128, FD], f16, tag="R1", name=f"R1_{e}")
                nc.vector.scalar_tensor_tensor(
                    R1[:], s2e[:], 1.0 / 3.0, w_[:], ALU.subtract, ALU.mult)
                R2f = scr.tile([128, FD], f16, tag="R2f", name=f"R2f_{e}")
                TT(nc.gpsimd, R2f[:], R1[:], z2ep2[:])
                R3f = scr.tile([128, FD], f16, tag="R3f", name=f"R3f_{e}")
                TT(nc.vector, R3f[:], Q1[:], z2epp[:])
                t1m = scr.tile([128, FD], f16, tag="t1m", name=f"t1m_{e}")
                nc.vector.scalar_tensor_tensor(
                    t1m[:], R2f[:], 3.0, R3f[:], ALU.mult, ALU.subtract)
                F1 = scr.tile([128, FD], f16, tag="F1", name=f"F1_{e}")
                TT(nc.vector, F1[:], t1m[:], Dz[:])
                DyNs = scr.tile([128, FD], f16, tag="DyNs", name=f"DyNs_{e}")
                nc.scalar.copy(DyNs[:], DyN[:])
                t2m = scr.tile([128, FD], f16, tag="t2m", name=f"t2m_{e}")
                nc.vector.scalar_tensor_tensor(
                    t2m[:], DzpN[:], 4.0, DyNs[:], ALU.mult, ALU.add)
                F2 = scr.tile([128, FD], f16, tag="F2", name=f"F2_{e}")
                TT(nc.gpsimd, F2[:], wpc[:], t2m[:])
                DypNs = scr.tile([128, FD], f16, tag="DypNs", name=f"DypNs_{e}")
                nc.scalar.copy(DypNs[:], DypN[:])
                t3m = scr.tile([128, FD], f16, tag="t3m", name=f"t3m_{e}")
                TT(nc.vector, t3m[:], Dzpp2[:], DypNs[:], ALU.add)
                F3 = scr.tile([128, FD], f16, tag="F3", name=f"F3_{e}")
                TT(nc.vector, F3[:], w_[:], t3m[:])

                vps = ps.tile([2, FD], f32, tag=trio[1], name=f"vps_{e}")
                nc.tensor.matmul(vps[:], smt["v6"][:, 0:2], F1[:], start=True,
                                 stop=False)
                nc.tensor.matmul(vps[:], smt["v6"][:, 2:4], F2[:], start=False,
                                 stop=False)
                nc.tensor.matmul(vps[:], smt["v6"][:, 4:6], F3[:], start=False,
                                 stop=True)
                ot = scr.tile([2, FD], f32, tag="ot", name=f"ot_{e}")
                nc.scalar.copy(ot[:], vps[:])
                nc.sync.dma_start(out_d.ap()[:, e * FD:(e + 1) * FD], ot[:])

    nc.compile()
    return nc


def _get_nc():
    if "nc" not in _CACHE:
        _CACHE["nc"] = _build()
    return _CACHE["nc"]


def kernel(**inputs):
    import concourse.bass_utils as bass_utils

    f = lambda k: np.asarray(inputs[k], np.float32)
    a, x, t = f("a"), f("x"), np.float32(inputs["t"])
    Wb, Wt1, bt1, Wt2, bt2 = f("Wb"), f("Wt1"), f("bt1"), f("Wt2"), f("bt2")
    Wt3, We1, be1, We2, be2, We3 = (
        f("Wt3"), f("We1"), f("be1"), f("We2"), f("be2"), f("We3"))

    h16 = lambda v: np.asarray(v, np.float32).astype(np.float16)
    def pair16(v):
        h = h16(v)
        return h, h16(np.asarray(v, np.float32) - h.astype(np.float32))

    w1 = Wt1[:, 0]
    c1b = (Wt1[:, 1] * t + bt1)[:, None]
    w1h, w1l = pair16(w1)
    w11 = np.stack([w1h, w1h, w1l, w1l])                       # [4,128]
    wt2t = np.ascontiguousarray(Wt2.T)
    wt2h, wt2l = pair16(wt2t)
    w2ah, w2al = pair16(wt2t * w1[:, None])
    w2b = h16(wt2t * (-2.0 * w1 ** 2)[:, None])
    w2c = h16(wt2t * (6.0 * w1 ** 3)[:, None])
    wt3h, wt3l = pair16(Wt3)

    p, q, v = We1[:, 0], We1[:, 1], We3[0]
    pq2 = np.zeros((1, 128), np.float16)
    pq2[0, 0:64] = h16(p)
    pq2[0, 64:128] = h16(q)

    blk = lambda M: np.block([[M, np.zeros_like(M)], [np.zeros_like(M), M]])
    We2T = We2.T
    e0 = h16(blk(We2T))
    eq = h16(blk(We2T * q[:, None]))
    ep = h16(blk(We2T * p[:, None]))
    v6 = np.zeros((128, 6), np.float16)
    for i in range(3):
        v6[0:64, 2 * i] = h16(2.0 * v)
        v6[64:128, 2 * i + 1] = h16(2.0 * v)
    sel4m = np.zeros((8, 4), np.float32)
    for j in range(4):
        sel4m[2 * j, j] = 1.0
        sel4m[2 * j + 1, j] = 1.0

    pkb = np.zeros((128, PKB_COLS), np.float16)
    for n_, arr in [("wt2h", wt2h), ("wt2l", wt2l), ("w2ah", w2ah),
                    ("w2al", w2al), ("w2b", w2b), ("w2c", w2c)]:
        pkb[:, _PKB[n_]:_PKB[n_] + 128] = arr
    pkc = np.zeros((128, PKC_COLS), np.float16)
    for n_, arr in [("wt3h", wt3h), ("wt3l", wt3l), ("e0", e0), ("eq", eq),
                    ("ep", ep)]:
        pkc[:, _PKC[n_]:_PKC[n_] + 128] = arr
    pkc[0:1, _PKC["pq2"]:_PKC["pq2"] + 128] = pq2
    pkc[:, _PKC["v6"]:_PKC["v6"] + 6] = v6
    pk32 = np.zeros((128, PK32_COLS), np.float32)
    pk32[:, 0] = c1b[:, 0]
    pk32[:, 1] = bt2
    pk32[:, 2] = np.concatenate([be1, be1])
    pk32[:, 3] = np.concatenate([be2, be2])
    pk32[0:8, 4:8] = sel4m

    smalls = {
        "w11": np.ascontiguousarray(w11),
        "pkb": np.ascontiguousarray(pkb),
        "pkc": np.ascontiguousarray(pkc),
        "pk32": np.ascontiguousarray(pk32),
    }

    in_maps = []
    for c in range(NCORES):
        blk_w = Wb[:, c * KSH:(c + 1) * KSH]                   # [128, 65536]
        tr = blk_w.T.reshape(NKT, 128, 128).transpose(1, 0, 2)  # [k1, kt, p]
        tr = tr.reshape(128, NCHUNK, KTC * 128).transpose(1, 0, 2)
        wsh = np.ascontiguousarray(h16(1024.0 * tr))           # [16,128,4096]
        ash = (a[c * KSH:(c + 1) * KSH] / 1024.0).reshape(NKT, 128).T  # [k1, kt]
        ah, al = pair16(ash)
        a2 = np.ascontiguousarray(np.stack([ah, al], axis=2))  # [128,512,2]
        xs = x[c * NPTS:(c + 1) * NPTS]
        xh, xl = pair16(xs)
        x4 = np.ascontiguousarray(np.stack([xh, xl, xh, xl]))  # [4,4096]
        im = {"w": wsh, "a2": a2, "x4": x4}
        im.update(smalls)
        in_maps.append(im)

    global _last_in_maps
    _last_in_maps = in_maps
    nc = _get_nc()
    res = bass_utils.run_bass_kernel_spmd(nc, in_maps, core_ids=list(range(NCORES)))
    outs = []
    for c in range(NCORES):
        o = res.results[c]["out"]          # [2, NPTS//2]
        outs.append(np.asarray(o).reshape(-1))
    return np.concatenate(outs).astype(np.float32)


# revision 18
# speedup vs baseline: 1.0734x; 1.0734x over previous
"""Bass/Trainium2 kernel for nn_HNO_37065567764989 (self-contained).

Strategy (8 NeuronCores, SPMD):
- Branch matvec b = Wb@a column-sharded 8 ways. Each core streams its 16MB
  shard as fp16 (W scaled by 2^10; a as an fp16 hi/lo stationary pair), two
  512KB DMAs per 1MB chunk across queues. 512B AllReduce combines partials.
- Nx=32768 points sharded 8 ways (4096/core). Trunk runs as 4 wide pairs
  (tiles f and f+4 share [128,1024] elementwise ops that write the energy
  movings directly). GpSimd carries only early-pair products plus the
  collective, so the mesh wait never blocks the trunk tail.
- EnergyNet first layer uses runtime outer-product stationaries S=c(x)p,
  c(x)q built on-device after the AllReduce -- no per-row extraction.
- Precision: t2/P1 flow as fp16 hi/lo pairs; t1/tp1, B/C stationaries and
  all product chains are single fp16 (mirror-validated 1.24e-2).
"""
import sys

for _p in ("/opt/trn_rl_repo",):
    if _p not in sys.path:
        sys.path.insert(0, _p)

import numpy as np

MP1, NX, P, HT, HE = 524288, 32768, 128, 128, 64
NCORES = 8
KSH = MP1 // NCORES        # 65536 contraction elems per core
NKT = KSH // 128           # 512 k-tiles
NCHUNK = 16
KTC = NKT // NCHUNK        # 32 k-tiles per chunk
NPTS = NX // NCORES        # 4096 points per core
FD = 512
WFD = 2 * FD               # wide pair width
NTRUNK = NPTS // FD        # 8 trunk tiles
NEN = NTRUNK // 2          # 4 energy tiles / trunk pairs

_PKB = {"wt2h": 0, "wt2l": 128, "w2ah": 256, "w2al": 384, "w2b": 512,
        "w2c": 640}
PKB_COLS = 768
_PKC = {"wt3h": 0, "wt3l": 128, "e0": 256, "eq": 384, "ep": 512,
        "pq2": 640, "v6": 768}
PKC_COLS = 774
_PK32 = {"c1b": 0, "bt2b": 1, "be1b2": 2, "be2b2": 3, "sel4m": 4}
PK32_COLS = 8

_CACHE = {}


def _build():
    import concourse.bacc as bacc
    import concourse.mybir as mybir
    from concourse import tile

    f32 = mybir.dt.float32
    f16 = mybir.dt.float16
    AF = mybir.ActivationFunctionType
    ALU = mybir.AluOpType

    nc = bacc.Bacc("TRN2", target_bir_lowering=False, debug=False,
                   num_devices=NCORES)

    w_d = nc.dram_tensor("w", [NCHUNK, 128, KTC * 128], f16, kind="ExternalInput")
    a_d = nc.dram_tensor("a2", [128, NKT, 2], f16, kind="ExternalInput")
    x_d = nc.dram_tensor("x4", [4, NPTS], f16, kind="ExternalInput")
    w11_d = nc.dram_tensor("w11", [4, 128], f16, kind="ExternalInput")
    pkb_d = nc.dram_tensor("pkb", [128, PKB_COLS], f16, kind="ExternalInput")
    pkc_d = nc.dram_tensor("pkc", [128, PKC_COLS], f16, kind="ExternalInput")
    pk32_d = nc.dram_tensor("pk32", [128, PK32_COLS], f32, kind="ExternalInput")
    out_d = nc.dram_tensor("out", [2, NPTS // 2], f32, kind="ExternalOutput")
    cc_in = nc.dram_tensor("cc_in", [128, 1], f32)
    cc_out = nc.dram_tensor("cc_out", [128, 1], f32, addr_space="Shared")

    def TT(eng, out, i0, i1, op=ALU.mult):
        eng.tensor_tensor(out, i0, i1, op)

    with tile.TileContext(nc) as tc:
        with (
            tc.tile_pool(name="smp", bufs=1) as smp,
            tc.tile_pool(name="persist", bufs=1) as persist,
            tc.tile_pool(name="wpool", bufs=4) as wpool,
            tc.tile_pool(name="scr", bufs=1) as scr,
            tc.tile_pool(name="ps8", bufs=1, space="PSUM") as ps,
        ):
            # ---- packed constant loads (6 DMA issues) ----
            x4 = smp.tile([4, NPTS], f16, name="x4t")
            nc.sync.dma_start(x4[:], x_d.ap())
            w11 = smp.tile([4, 128], f16, name="w11t")
            nc.sync.dma_start(w11[:], w11_d.ap())
            pk32 = smp.tile([128, PK32_COLS], f32, name="pk32t")
            nc.sync.dma_start(pk32[:], pk32_d.ap())
            a2 = smp.tile([128, NKT, 2], f16, name="a2t")
            nc.sync.dma_start(a2[:], a_d.ap())
            pkb = smp.tile([128, PKB_COLS], f16, name="pkbt")
            nc.sync.dma_start(pkb[:], pkb_d.ap())
            pkc = smp.tile([128, PKC_COLS], f16, name="pkct")
            nc.sync.dma_start(pkc[:], pkc_d.ap())

            smt = {"w11": w11[:]}
            for n_, c0 in _PKB.items():
                smt[n_] = pkb[:, c0:c0 + 128]
            for n_, c0 in _PKC.items():
                if n_ == "v6":
                    smt[n_] = pkc[:, c0:c0 + 6]
                elif n_ == "pq2":
                    smt[n_] = pkc[0:1, c0:c0 + 128]
                else:
                    smt[n_] = pkc[:, c0:c0 + 128]
            for n_, c0 in _PK32.items():
                if n_ == "sel4m":
                    smt[n_] = pk32[0:8, c0:c0 + 4]
                else:
                    smt[n_] = pk32[:, c0:c0 + 1]

            # ---- trunk layer-1 z1 matmuls (pairs f, f+4) ----
            # z1 borrows pB/pC/pBC banks so zA/zB/aA/aB stay free for the
            # layer-2 wave to start as soon as l1 of pair 0 is done.
            zpair = [("zA", "zB"), ("aA", "aB")]
            z1tags = ["pB", "pC", "pBC"]
            z1ps = {}
            zi = 0
            for j in range(NEN):
                for hx, f in enumerate((j, j + 4)):
                    cs = slice(f * FD, (f + 1) * FD)
                    z1 = ps.tile([128, FD], f32, tag=z1tags[zi % 3], name=f"z1_{f}")
                    zi += 1
                    nc.tensor.matmul(z1[:], smt["w11"], x4[:, cs], start=True,
                                     stop=True)
                    z1ps[f] = z1

            # ---- trunk layer-1 elementwise (wide pairs, single-fp16 t1/tp1) --
            l1 = {}
            for j in range(NEN):
                t1f = scr.tile([128, WFD], f32, tag="t1f", name=f"t1f_{j}")
                for hx, f in enumerate((j, j + 4)):
                    hs = slice(hx * FD, (hx + 1) * FD)
                    nc.scalar.activation(t1f[:, hs], z1ps[f][:], AF.Tanh,
                                         bias=smt["c1b"])
                t1h = persist.tile([128, WFD], f16, tag=f"t1h_{j % 2}", name=f"t1h_{j}")
                nc.scalar.copy(t1h[:], t1f[:])
                s1 = scr.tile([128, WFD], f32, tag="s1", name=f"s1_{j}")
                nc.scalar.square(s1[:], t1f[:])
                tp1f = scr.tile([128, WFD], f32, tag="tp1f", name=f"tp1f_{j}")
                nc.scalar.activation(tp1f[:], s1[:], AF.Copy, bias=1.0, scale=-1.0)
                tp1h = persist.tile([128, WFD], f16, tag=f"tp1h_{j % 2}", name=f"tp1h_{j}")
                nc.scalar.copy(tp1h[:], tp1f[:])
                g2m = persist.tile([128, WFD], f16, tag=f"g2m_{j % 2}", name=f"g2m_{j}")
                TT(nc.vector, g2m[:], t1h[:], tp1h[:])
                # g3m = (tp1-2/3)*tp1 = -(s1-1/3)*tp1; sign folded into H below
                g3m = persist.tile([128, WFD], f16, tag=f"g3m_{j % 2}", name=f"g3m_{j}")
                nc.vector.scalar_tensor_tensor(
                    g3m[:], tp1h[:], 2.0 / 3.0, tp1h[:], ALU.subtract, ALU.mult)
                l1[j] = (t1h, tp1h, g2m, g3m)

            # ---- trunk layer-2 wave (wide pairs) ----
            sh = {}
            for j in range(NEN):
                t1h, tp1h, g2m, g3m = l1[j]
                shj = tuple(
                    persist.tile([128, WFD], f16, tag=f"sh{nm}_{j}",
                                 name=f"sh_{nm}_{j}")
                    for nm in ("t2h", "t2l", "P1h", "P1l", "ux2", "ux3"))
                t2h_s, t2l_s, P1h_s, P1l_s, ux2_s, ux3_s = shj
                sh[j] = shj
                ge = nc.gpsimd if j < 2 else nc.vector

                zw, aw = [], []
                for hx in range(2):
                    hs = slice(hx * FD, (hx + 1) * FD)
                    z2 = ps.tile([128, FD], f32, tag=zpair[0][hx], name=f"z2_{j}{hx}")
                    nc.tensor.matmul(z2[:], smt["wt2h"], t1h[:, hs], start=True,
                                     stop=False)
                    nc.tensor.matmul(z2[:], smt["wt2l"], t1h[:, hs], start=False,
                                     stop=True)
                    zw.append(z2)
                    A = ps.tile([128, FD], f32, tag=zpair[1][hx], name=f"A_{j}{hx}")
                    nc.tensor.matmul(A[:], smt["w2ah"], tp1h[:, hs], start=True,
                                     stop=False)
                    nc.tensor.matmul(A[:], smt["w2al"], tp1h[:, hs], start=False,
                                     stop=True)
                    aw.append(A)

                Bc = scr.tile([128, WFD], f16, tag="Bc", name=f"Bc_{j}")
                Cc = scr.tile([128, WFD], f16, tag="Cc", name=f"Cc_{j}")
                A2c = scr.tile([128, WFD], f16, tag="A2c", name=f"A2c_{j}")
                Acp = scr.tile([128, WFD], f16, tag="Acp", name=f"Acp_{j}")
                t2f = scr.tile([128, WFD], f32, tag="t2f", name=f"t2f_{j}")
                for hx in range(2):
                    hs = slice(hx * FD, (hx + 1) * FD)
                    nc.scalar.activation(t2f[:, hs], zw[hx][:], AF.Tanh,
                                         bias=smt["bt2b"])
                    nc.scalar.square(A2c[:, hs], aw[hx][:])
                    nc.scalar.copy(Acp[:, hs], aw[hx][:])
                    B = ps.tile([128, FD], f32, tag="pB", name=f"B_{j}{hx}")
                    nc.tensor.matmul(B[:], smt["w2b"], g2m[:, hs], start=True,
                                     stop=True)
                    nc.scalar.copy(Bc[:, hs], B[:])
                    C = ps.tile([128, FD], f32, tag="pC", name=f"C_{j}{hx}")
                    nc.tensor.matmul(C[:], smt["w2c"], g3m[:, hs], start=True,
                                     stop=True)
                    nc.scalar.copy(Cc[:, hs], C[:])

                nc.scalar.copy(t2h_s[:], t2f[:])
                TT(nc.gpsimd, t2l_s[:], t2f[:], t2h_s[:], ALU.subtract)
                s2 = scr.tile([128, WFD], f32, tag="s2", name=f"s2_{j}")
                nc.scalar.square(s2[:], t2f[:])
                tp2 = scr.tile([128, WFD], f32, tag="tp2", name=f"tp2_{j}")
                nc.vector.tensor_scalar(tp2[:], s2[:], -1.0, 1.0, ALU.mult, ALU.add)
                tp2c = scr.tile([128, WFD], f16, tag="tp2c", name=f"tp2c_{j}")
                nc.scalar.activation(tp2c[:], s2[:], AF.Copy, bias=1.0, scale=-1.0)
                P1f = scr.tile([128, WFD], f32, tag="P1f", name=f"P1f_{j}")
                for hx in range(2):
                    hs = slice(hx * FD, (hx + 1) * FD)
                    TT(nc.vector, P1f[:, hs], tp2[:, hs], aw[hx][:])
                nc.scalar.copy(P1h_s[:], P1f[:])
                TT(nc.gpsimd, P1l_s[:], P1f[:], P1h_s[:], ALU.subtract)

                T1 = scr.tile([128, WFD], f16, tag="T1", name=f"T1_{j}")
                TT(ge, T1[:], t2h_s[:], A2c[:])
                E = scr.tile([128, WFD], f16, tag="E", name=f"E_{j}")
                nc.vector.scalar_tensor_tensor(
                    E[:], T1[:], -2.0, Bc[:], ALU.mult, ALU.add)
                TT(nc.vector, ux2_s[:], tp2c[:], E[:])
                A3 = scr.tile([128, WFD], f16, tag="A3", name=f"A3_{j}")
                TT(ge, A3[:], A2c[:], Acp[:])
                G1 = scr.tile([128, WFD], f16, tag="G1", name=f"G1_{j}")
                nc.vector.scalar_tensor_tensor(
                    G1[:], tp2c[:], 2.0 / 3.0, A3[:], ALU.subtract, ALU.mult)
                G2 = scr.tile([128, WFD], f16, tag="G2", name=f"G2_{j}")
                TT(ge, G2[:], t2h_s[:], Acp[:])
                G3 = scr.tile([128, WFD], f16, tag="G3", name=f"G3_{j}")
                TT(ge, G3[:], G2[:], Bc[:])
                D = scr.tile([128, WFD], f16, tag="Dd", name=f"D_{j}")
                TT(nc.vector, D[:], G1[:], G3[:], ALU.add)
                # Cc holds -C_true (g3m sign-flip): H = -6*D - Cc = -6*D + C
                H = scr.tile([128, WFD], f16, tag="Hh", name=f"H_{j}")
                nc.vector.scalar_tensor_tensor(
                    H[:], D[:], -6.0, Cc[:], ALU.mult, ALU.subtract)
                TT(nc.vector, ux3_s[:], tp2c[:], H[:])

            # ---- matvec: stream W shard (2 DMA splits per 1MB chunk) ----
            b8 = ps.tile([8, FD], f32, tag="pMV", name="b8")
            half = KTC * 64
            for i in range(NCHUNK):
                wch = wpool.tile([128, KTC * 128], f16, tag="wch", name="wch")
                nc.sync.dma_start(wch[:, 0:half], w_d.ap()[i][:, 0:half])
                nc.sync.dma_start(wch[:, half:], w_d.ap()[i][:, half:])
                for g in range(KTC // 4):
                    nc.tensor.matmul(
                        b8[:], a2[:, i * KTC + 4 * g:i * KTC + 4 * (g + 1), :],
                        wch[:, g * 512:(g + 1) * 512],
                        start=(i == 0 and g == 0),
                        stop=(i == NCHUNK - 1 and g == KTC // 4 - 1),
                    )

            # ---- local reduce + AllReduce (high priority: the mesh gates
            # the whole energy phase, so these must not sit behind the
            # trunk wave in the engine queues) ----
            with tc.high_priority():
                b8sb = smp.tile([8, FD], f32, name="b8sb")
                nc.scalar.copy(b8sb[:], b8[:])
                bcol = ps.tile([128, 1], f32, tag="pBC", name="bcol")
                for j in range(4):
                    nc.tensor.matmul(bcol[:], b8sb[:, j * 128:(j + 1) * 128],
                                     smt["sel4m"][:, j:j + 1],
                                     start=(j == 0), stop=(j == 3))
                b_loc = smp.tile([128, 1], f32, name="bloc")
                nc.scalar.copy(b_loc[:], bcol[:])
                nc.sync.dma_start(cc_in.ap(), b_loc[:])
                nc.gpsimd.collective_compute(
                    "AllReduce", ALU.add,
                    replica_groups=[list(range(NCORES))],
                    ins=[cc_in.ap()], outs=[cc_out.ap()],
                )
                b_ar = smp.tile([128, 1], f32, name="bar")
                nc.sync.dma_start(b_ar[:], cc_out.ap())

            # ---- b -> c -> outer-product stationaries S = c(x)p, c(x)q ----
            with tc.high_priority():
                b16 = smp.tile([128, 1], f16, name="b16")
                nc.scalar.copy(b16[:], b_ar[:])
                c0p = ps.tile([1, 128], f32, tag="pBC", name="c0p")
                nc.tensor.matmul(c0p[:], b16[:], smt["wt3h"], start=True, stop=False)
                nc.tensor.matmul(c0p[:], b16[:], smt["wt3l"], start=False, stop=True)
                c0sb = smp.tile([1, 128], f16, name="c0sb")
                nc.scalar.copy(c0sb[:], c0p[:])
                scpq_p = ps.tile([128, 128], f32, tag="pBC", name="scpq_p")
                nc.tensor.matmul(scpq_p[:], c0sb[:], smt["pq2"], start=True, stop=True)
                Scpq16 = smp.tile([128, 128], f16, name="Scpq16")
                nc.scalar.copy(Scpq16[:], scpq_p[:])
            Sp16, Sq16 = Scpq16[:, 0:64], Scpq16[:, 64:128]

            # ---- energy phase ----
            for e in range(NEN):
                t2h_s, t2l_s, P1h_s, P1l_s, ux2_s, ux3_s = sh[e]
                trio = [["zA", "zB", "aA"], ["aB", "pB", "pC"]][e % 2]
                dzt, dyt = ("pBC", "pMV") if e % 2 == 0 else ("pMV", "pBC")

                z1e = ps.tile([128, FD], f32, tag=trio[0], name=f"z1e_{e}")
                z1p = ps.tile([128, FD], f32, tag=trio[1], name=f"z1p_{e}")
                z1pp = ps.tile([128, FD], f32, tag=trio[2], name=f"z1pp_{e}")
                for hx in range(2):
                    hs = slice(hx * FD, (hx + 1) * FD)
                    rs = slice(hx * 64, (hx + 1) * 64)
                    nc.tensor.matmul(z1e[rs, :], Sp16, t2h_s[:, hs], start=True,
                                     stop=False)
                    nc.tensor.matmul(z1e[rs, :], Sp16, t2l_s[:, hs], start=False,
                                     stop=False)
                    nc.tensor.matmul(z1e[rs, :], Sq16, P1h_s[:, hs], start=False,
                                     stop=False)
                    nc.tensor.matmul(z1e[rs, :], Sq16, P1l_s[:, hs], start=False,
                                     stop=True)
                    nc.tensor.matmul(z1p[rs, :], Sp16, P1h_s[:, hs], start=True,
                                     stop=False)
                    nc.tensor.matmul(z1p[rs, :], Sp16, P1l_s[:, hs], start=False,
                                     stop=False)
                    nc.tensor.matmul(z1p[rs, :], Sq16, ux2_s[:, hs], start=False,
                                     stop=True)
                    nc.tensor.matmul(z1pp[rs, :], Sp16, ux2_s[:, hs], start=True,
                                     stop=False)
                    nc.tensor.matmul(z1pp[rs, :], Sq16, ux3_s[:, hs], start=False,
                                     stop=True)

                t1ef = scr.tile([128, FD], f32, tag="t1ef", name=f"t1ef_{e}")
                nc.scalar.activation(t1ef[:], z1e[:], AF.Tanh, bias=smt["be1b2"])
                t1eh = scr.tile([128, FD], f16, tag="t1eh", name=f"t1eh_{e}")
                nc.scalar.copy(t1eh[:], t1ef[:])
                z1psb = scr.tile([128, FD], f16, tag="z1psb", name=f"z1psb_{e}")
                nc.scalar.copy(z1psb[:], z1p[:])
                z1ppsb = scr.tile([128, FD], f16, tag="z1ppsb", name=f"z1ppsb_{e}")
                nc.scalar.copy(z1ppsb[:], z1pp[:])
                s1e = scr.tile([128, FD], f16, tag="s1e", name=f"s1e_{e}")
                nc.scalar.square(s1e[:], t1ef[:])
                m_ = scr.tile([128, FD], f16, tag="m_", name=f"m_{e}")
                nc.scalar.activation(m_[:], s1e[:], AF.Copy, bias=1.0, scale=-1.0)
                z1p2 = scr.tile([128, FD], f16, tag="z1p2", name=f"z1p2_{e}")
                TT(nc.gpsimd, z1p2[:], z1psb[:], z1psb[:])
                N1 = scr.tile([128, FD], f16, tag="N1", name=f"N1_{e}")
                TT(nc.vector, N1[:], t1eh[:], m_[:])
                a1p = scr.tile([128, FD], f16, tag="a1p", name=f"a1p_{e}")
                TT(nc.vector, a1p[:], m_[:], z1psb[:])
                N2 = scr.tile([128, FD], f16, tag="N2", name=f"N2_{e}")
                TT(nc.gpsimd, N2[:], N1[:], z1p2[:])
                N3 = scr.tile([128, FD], f16, tag="N3", name=f"N3_{e}")
                TT(nc.vector, N3[:], m_[:], z1ppsb[:])
                zin = scr.tile([128, FD], f16, tag="zin", name=f"zin_{e}")
                nc.vector.scalar_tensor_tensor(
                    zin[:], N2[:], -2.0, N3[:], ALU.mult, ALU.add)
                mpc = scr.tile([128, FD], f16, tag="mpc", name=f"mpc_{e}")
                TT(nc.vector, mpc[:], N1[:], z1psb[:])
                O1 = scr.tile([128, FD], f16, tag="O1", name=f"O1_{e}")
                nc.vector.scalar_tensor_tensor(
                    O1[:], s1e[:], 1.0 / 3.0, m_[:], ALU.subtract, ALU.mult)
                O2f = scr.tile([128, FD], f16, tag="O2f", name=f"O2f_{e}")
                TT(nc.gpsimd, O2f[:], O1[:], z1p2[:])
                O3f = scr.tile([128, FD], f16, tag="O3f", name=f"O3f_{e}")
                TT(nc.vector, O3f[:], N1[:], z1ppsb[:])
                O2m = scr.tile([128, FD], f16, tag="O2m", name=f"O2m_{e}")
                nc.vector.scalar_tensor_tensor(
                    O2m[:], O2f[:], 3.0, O3f[:], ALU.mult, ALU.subtract)

                z2e = ps.tile([128, FD], f32, tag=trio[0], name=f"z2e_{e}")
                nc.tensor.matmul(z2e[:], smt["e0"], t1eh[:], start=True, stop=True)
                z2ep = ps.tile([128, FD], f32, tag=trio[1], name=f"z2ep_{e}")
                nc.tensor.matmul(z2ep[:], smt["e0"], a1p[:], start=True, stop=True)
                z2epp = ps.tile([128, FD], f32, tag=trio[2], name=f"z2epp_{e}")
                nc.tensor.matmul(z2epp[:], smt["e0"], zin[:], start=True, stop=True)
                Dz = ps.tile([128, FD], f32, tag=dzt, name=f"Dz_{e}")
                nc.tensor.matmul(Dz[:], smt["eq"], m_[:], start=True, stop=True)
                DyN = ps.tile([128, FD], f32, tag=dyt, name=f"DyN_{e}")
                nc.tensor.matmul(DyN[:], smt["ep"], m_[:], start=True, stop=True)
                DzpN = ps.tile([128, FD], f32, tag=trio[0], name=f"DzpN_{e}")
                nc.tensor.matmul(DzpN[:], smt["eq"], mpc[:], start=True, stop=True)
                DypN = ps.tile([128, FD], f32, tag=trio[1], name=f"DypN_{e}")
                nc.tensor.matmul(DypN[:], smt["ep"], mpc[:], start=True, stop=True)
                Dzpp2 = ps.tile([128, FD], f32, tag=trio[2], name=f"Dzpp2_{e}")
                nc.tensor.matmul(Dzpp2[:], smt["eq"], O2m[:], start=True, stop=True)

                t2e = scr.tile([128, FD], f16, tag="t2e", name=f"t2e_{e}")
                nc.scalar.activation(t2e[:], z2e[:], AF.Tanh, bias=smt["be2b2"])
                s2e = scr.tile([128, FD], f16, tag="s2e", name=f"s2e_{e}")
                nc.scalar.square(s2e[:], t2e[:])
                w_ = scr.tile([128, FD], f16, tag="w_", name=f"w_{e}")
                nc.scalar.activation(w_[:], s2e[:], AF.Copy, bias=1.0, scale=-1.0)
                z2ep16 = scr.tile([128, FD], f16, tag="z2ep16", name=f"z2ep16_{e}")
                nc.scalar.copy(z2ep16[:], z2ep[:])
                z2ep2 = scr.tile([128, FD], f16, tag="z2ep2", name=f"z2ep2_{e}")
                TT(nc.gpsimd, z2ep2[:], z2ep16[:], z2ep16[:])
                Q1 = scr.tile([128, FD], f16, tag="Q1", name=f"Q1_{e}")
                TT(nc.vector, Q1[:], t2e[:], w_[:])
                wpc = scr.tile([128, FD], f16, tag="wpc", name=f"wpc_{e}")
                TT(nc.vector, wpc[:], Q1[:], z2ep16[:])
                R1 = scr.tile([128, FD], f16, tag="R1", name=f"R1_{e}")
                nc.vector.scalar_tensor_tensor(
                    R1[:], s2e[:], 1.0 / 3.0, w_[:], ALU.subtract, ALU.mult)
                R2f = scr.tile([128, FD], f16, tag="R2f", name=f"R2f_{e}")
                TT(nc.gpsimd, R2f[:], R1[:], z2ep2[:])
                R3f = scr.tile([128, FD], f16, tag="R3f", name=f"R3f_{e}")
                TT(nc.vector, R3f[:], Q1[:], z2epp[:])
                t1m = scr.tile([128, FD], f16, tag="t1m", name=f"t1m_{e}")
                nc.vector.scalar_tensor_tensor(
                    t1m[:], R2f[:], 3.0, R3f[:], ALU.mult, ALU.subtract)
                F1 = scr.tile([128, FD], f16, tag="F1", name=f"F1_{e}")
                TT(nc.vector, F1[:], t1m[:], Dz[:])
                DyNs = scr.tile([128, FD], f16, tag="DyNs", name=f"DyNs_{e}")
                nc.scalar.copy(DyNs[:], DyN[:])
                t2m = scr.tile([128, FD], f16, tag="t2m", name=f"t2m_{e}")
                nc.vector.scalar_tensor_tensor(
                    t2m[:], DzpN[:], 4.0, DyNs[:], ALU.mult, ALU.add)
                F2 = scr.tile([128, FD], f16, tag="F2", name=f"F2_{e}")
                TT(nc.gpsimd, F2[:], wpc[:], t2m[:])
                DypNs = scr.tile([128, FD], f16, tag="DypNs", name=f"DypNs_{e}")
                nc.scalar.copy(DypNs[:], DypN[:])
                t3m = scr.tile([128, FD], f16, tag="t3m", name=f"t3m_{e}")
                TT(nc.vector, t3m[:], Dzpp2[:], DypNs[:], ALU.add)
                F3 = scr.tile([128, FD], f16, tag="F3", name=f"F3_{e}")
                TT(nc.vector, F3[:], w_[:], t3m[:])

                vps = ps.tile([2, FD], f32, tag=trio[1], name=f"vps_{e}")
                nc.tensor.matmul(vps[:], smt["v6"][:, 0:2], F1[:], start=True,
                                 stop=False)
                nc.tensor.matmul(vps[:], smt["v6"][:, 2:4], F2[:], start=False,
                                 stop=False)
                nc.tensor.matmul(vps[:], smt["v6"][:, 4:6], F3[:], start=False,
                                 stop=True)
                ot = scr.tile([2, FD], f32, tag="ot", name=f"ot_{e}")
                nc.scalar.copy(ot[:], vps[:])
                nc.sync.dma_start(out_d.ap()[:, e * FD:(e + 1) * FD], ot[:])

    nc.compile()
    return nc


def _get_nc():
    if "nc" not in _CACHE:
        _CACHE["nc"] = _build()
    return _CACHE["nc"]


def kernel(**inputs):
    import concourse.bass_utils as bass_utils

    f = lambda k: np.asarray(inputs[k], np.float32)
    a, x, t = f("a"), f("x"), np.float32(inputs["t"])
    Wb, Wt1, bt1, Wt2, bt2 = f("Wb"), f("Wt1"), f("bt1"), f("Wt2"), f("bt2")
    Wt3, We1, be1, We2, be2, We3 = (
        f("Wt3"), f("We1"), f("be1"), f("We2"), f("be2"), f("We3"))

    h16 = lambda v: np.asarray(v, np.float32).astype(np.float16)
    def pair16(v):
        h = h16(v)
        return h, h16(np.asarray(v, np.float32) - h.astype(np.float32))

    w1 = Wt1[:, 0]
    c1b = (Wt1[:, 1] * t + bt1)[:, None]
    w1h, w1l = pair16(w1)
    w11 = np.stack([w1h, w1h, w1l, w1l])                       # [4,128]
    wt2t = np.ascontiguousarray(Wt2.T)
    wt2h, wt2l = pair16(wt2t)
    w2ah, w2al = pair16(wt2t * w1[:, None])
    w2b = h16(wt2t * (-2.0 * w1 ** 2)[:, None])
    w2c = h16(wt2t * (6.0 * w1 ** 3)[:, None])
    wt3h, wt3l = pair16(Wt3)

    p, q, v = We1[:, 0], We1[:, 1], We3[0]
    pq2 = np.zeros((1, 128), np.float16)
    pq2[0, 0:64] = h16(p)
    pq2[0, 64:128] = h16(q)

    blk = lambda M: np.block([[M, np.zeros_like(M)], [np.zeros_like(M), M]])
    We2T = We2.T
    e0 = h16(blk(We2T))
    eq = h16(blk(We2T * q[:, None]))
    ep = h16(blk(We2T * p[:, None]))
    v6 = np.zeros((128, 6), np.float16)
    for i in range(3):
        v6[0:64, 2 * i] = h16(2.0 * v)
        v6[64:128, 2 * i + 1] = h16(2.0 * v)
    sel4m = np.zeros((8, 4), np.float32)
    for j in range(4):
        sel4m[2 * j, j] = 1.0
        sel4m[2 * j + 1, j] = 1.0

    pkb = np.zeros((128, PKB_COLS), np.float16)
    for n_, arr in [("wt2h", wt2h), ("wt2l", wt2l), ("w2ah", w2ah),
                    ("w2al", w2al), ("w2b", w2b), ("w2c", w2c)]:
        pkb[:, _PKB[n_]:_PKB[n_] + 128] = arr
    pkc = np.zeros((128, PKC_COLS), np.float16)
    for n_, arr in [("wt3h", wt3h), ("wt3l", wt3l), ("e0", e0), ("eq", eq),
                    ("ep", ep)]:
        pkc[:, _PKC[n_]:_PKC[n_] + 128] = arr
    pkc[0:1, _PKC["pq2"]:_PKC["pq2"] + 128] = pq2
    pkc[:, _PKC["v6"]:_PKC["v6"] + 6] = v6
    pk32 = np.zeros((128, PK32_COLS), np.float32)
    pk32[:, 0] = c1b[:, 0]
    pk32[:, 1] = bt2
    pk32[:, 2] = np.concatenate([be1, be1])
    pk32[:, 3] = np.concatenate([be2, be2])
    pk32[0:8, 4:8] = sel4m

    smalls = {
        "w11": np.ascontiguousarray(w11),
        "pkb": np.ascontiguousarray(pkb),
        "pkc": np.ascontiguousarray(pkc),
        "pk32": np.ascontiguousarray(pk32),
    }

    in_maps = []
    for c in range(NCORES):
        blk_w = Wb[:, c * KSH:(c + 1) * KSH]                   # [128, 65536]
        tr = blk_w.T.reshape(NKT, 128, 128).transpose(1, 0, 2)  # [k1, kt, p]
        tr = tr.reshape(128, NCHUNK, KTC * 128).transpose(1, 0, 2)
        wsh = np.ascontiguousarray(h16(1024.0 * tr))           # [16,128,4096]
        ash = (a[c * KSH:(c + 1) * KSH] / 1024.0).reshape(NKT, 128).T  # [k1, kt]
        ah, al = pair16(ash)
        a2 = np.ascontiguousarray(np.stack([ah, al], axis=2))  # [128,512,2]
        xs = x[c * NPTS:(c + 1) * NPTS]
        xh, xl = pair16(xs)
        x4 = np.ascontiguousarray(np.stack([xh, xl, xh, xl]))  # [4,4096]
        im = {"w": wsh, "a2": a2, "x4": x4}
        im.update(smalls)
        in_maps.append(im)

    global _last_in_maps
    _last_in_maps = in_maps
    nc = _get_nc()
    res = bass_utils.run_bass_kernel_spmd(nc, in_maps, core_ids=list(range(NCORES)))
    outs = []
    for c in range(NCORES):
        o = res.results[c]["out"]          # [2, NPTS//2]
        outs.append(np.asarray(o).reshape(-1))
    return np.concatenate(outs).astype(np.float32)
